# revision 33
# baseline (speedup 1.0000x reference)
"""Trainium2 Bass kernel for nn_DiVimEncoder (Vision-Mamba encoder), v4.

Sharding: 8 cores = batch(2) x d_inner-quarter(4). Each core runs the full
token stream feature-major; each core owns a 96-channel quarter of
d_inner. The d_model-wide lp/rmsnorm chain is replicated inside a batch
group; conv+in_proj, xproj and the output projection contract only the
core's own quarter, with fp16 AllReduce collectives combining the
quarter-partial xproj (44 x T) and output (192 x T) products.

v4 structure:
  - all matmul operands fp16 (1 cycle/row on PE)
  - software-pipelined emission per chunk: PRE_MM(c+1) [matmuls + silu]
    -> PRE_BC(c) [consume xproj AllReduce: dt/ladder/B/C] -> POST1(c-1)
    -> POST2(c-2) -> dBuM(c) -> SCANS(c); collectives get a full step
    of latency hiding
  - depthwise conv folded into in_proj as shifted matmul taps (8-tap
    accumulation for the core's own quarter only)
  - silu entirely on Act (exp/ln/exp sigmoid) + one Pool STT mul reading
    PSUM directly; lp/conv biases folded into Act bias APs
  - residual stream s and nrm kept in SBUF across layers
  - scan tree-reduce + HC on DVE; dA ladder split Act/DVE/Pool
"""
import numpy as np
from contextlib import ExitStack

import concourse.bass as bass
import concourse.bacc as bacc
import concourse.tile as tile
import concourse.mybir as mybir
from concourse.bass_utils import run_bass_kernel_spmd

F32 = mybir.dt.float32
F16 = mybir.dt.float16
AF = mybir.ActivationFunctionType
OP = mybir.AluOpType

D_MODEL = 192
DEPTH = 12
D_INNER = 384
DS = 16
D_CONV = 4
DT_RANK = 12
EPS = 1e-5
N = 2304
DQ = 96
TC = 384
NCH = N // TC
NCORES = 8
GROUPS = [[0, 1, 2, 3], [4, 5, 6, 7]]

LAD_EXP_S = [0, 1, 2, 3, 7]
LAD_MUL_S = [(4, 0, 3), (5, 1, 3), (6, 2, 3), (8, 0, 7), (9, 1, 7),
             (10, 2, 7), (11, 3, 7), (12, 4, 7), (13, 5, 7), (14, 6, 7),
             (15, 7, 7)]

_CACHE = {}

_gat_patched = False


def _patch_act_tables():
    global _gat_patched
    if _gat_patched:
        return
    from concourse import hw_specs
    real = hw_specs.get_activation_tables

    def patched(arch):
        t = dict(real(arch))
        keep_name = "natural_log_exp_and_others"
        keep = t[keep_name]
        return {name: (funcs if name == keep_name else funcs - keep)
                for name, funcs in t.items()}

    bacc.get_activation_tables = patched
    _gat_patched = True


def _build(A_vals, depth=DEPTH, n_tok=N, sim_mode=False):
    _patch_act_tables()
    chunks = [(c, min(c + TC, n_tok)) for c in range(0, n_tok, TC)]
    nc = bacc.Bacc("TRN2", target_bir_lowering=False, debug=False,
                   enable_asserts=True, num_devices=NCORES)

    s0_d = nc.dram_tensor("s0", [DQ, 2, n_tok], F16, kind="ExternalInput")
    lpT_d = nc.dram_tensor("lpT", [depth, DQ, 2, 2, 2, DQ], F16, kind="ExternalInput")
    lpb_d = nc.dram_tensor("lpb", [depth, DQ, 2], F32, kind="ExternalInput")
    cvip_d = nc.dram_tensor("cvip", [depth, DQ, 2, D_CONV, DQ], F16, kind="ExternalInput")
    ipz_d = nc.dram_tensor("ipz", [depth, DQ, 2, DQ], F16, kind="ExternalInput")
    cvb_d = nc.dram_tensor("cv_b", [depth, DQ, 2], F32, kind="ExternalInput")
    xpT_d = nc.dram_tensor("xpT", [depth, DQ, 44], F16, kind="ExternalInput")
    dtT_d = nc.dram_tensor("dtT", [depth, DT_RANK, 3, 128], F16, kind="ExternalInput")
    dtb_d = nc.dram_tensor("dt_b", [depth, 128, 3], F32, kind="ExternalInput")
    apk_d = nc.dram_tensor("Apk", [depth, 128, 4], F32, kind="ExternalInput")
    epk_d = nc.dram_tensor("Epk", [depth, 128, 1], F32, kind="ExternalInput")
    r32_d = nc.dram_tensor("red32", [128, 32], F16, kind="ExternalInput")
    dtT9_d = nc.dram_tensor("dtT9", [depth, DT_RANK, DQ], F16, kind="ExternalInput")
    dtb9_d = nc.dram_tensor("dtb9", [depth, DQ, 1], F32, kind="ExternalInput")
    Dsm_d = nc.dram_tensor("Dssm", [depth, DQ, 1], F32, kind="ExternalInput")
    owT_d = nc.dram_tensor("owT", [depth, DQ, 2, DQ], F16, kind="ExternalInput")
    nfw_d = nc.dram_tensor("nfw", [DQ, 2], F32, kind="ExternalInput")
    out_d = nc.dram_tensor("out_s", [2, DQ, n_tok], F16, kind="ExternalOutput")

    with tile.TileContext(nc) as tc, ExitStack() as ctx:
        consts = ctx.enter_context(tc.tile_pool(name="consts", bufs=1))
        sres = ctx.enter_context(tc.tile_pool(name="sres", bufs=1))
        nrmp = ctx.enter_context(tc.tile_pool(name="nrmp", bufs=2))
        wts = ctx.enter_context(tc.tile_pool(name="wts", bufs=1))
        ck2 = ctx.enter_context(tc.tile_pool(name="ck2", bufs=2))
        ck1 = ctx.enter_context(tc.tile_pool(name="ck1", bufs=1))
        scn = ctx.enter_context(tc.tile_pool(name="scn", bufs=18))
        big = ctx.enter_context(tc.tile_pool(name="big", bufs=2))
        pa = ctx.enter_context(tc.tile_pool(name="pa", bufs=3, space="PSUM"))
        pb = ctx.enter_context(tc.tile_pool(name="pb", bufs=1, space="PSUM"))
        dram = ctx.enter_context(tc.tile_pool(name="dram", bufs=3, space="DRAM"))

        ones_r = consts.tile([1, DQ], F16)
        nc.gpsimd.memset(ones_r[:], 1.0)
        ones_ch = consts.tile([DQ, 1], F16)
        nc.gpsimd.memset(ones_ch[:], 1.0)
        nfw = consts.tile([DQ, 2], F32)
        nc.sync.dma_start(nfw[:], nfw_d.ap())
        epsc = consts.tile([1, 1], F32)
        nc.gpsimd.memset(epsc[:], EPS)
        red32 = consts.tile([128, 32], F16)
        nc.sync.dma_start(red32[:], r32_d.ap())

        s_tiles = []
        for pi in range(2):
            st = sres.tile([DQ, 2, 1 + n_tok], F16, tag=f"s{pi}", name=f"s{pi}")
            nc.gpsimd.memset(st[:, :, 0:1], 0.0)
            s_tiles.append(st)
        nc.sync.dma_start(s_tiles[0][:, :, 1:1 + n_tok], s0_d.ap())

        W = {}
        P = {}

        def load_weights(li):
            w = {}
            w['lpT'] = wts.tile([DQ, 2, 2, 2, DQ], F16, tag="lpT", name="lpT", bufs=2)
            nc.sync.dma_start(w['lpT'][:], lpT_d.ap()[li])
            w['lpb'] = wts.tile([DQ, 2], F32, tag="lpb", name="lpb", bufs=2)
            nc.sync.dma_start(w['lpb'][:], lpb_d.ap()[li])
            w['cvip'] = wts.tile([DQ, 2, D_CONV, DQ], F16, tag="cvip", name="cvip", bufs=2)
            nc.sync.dma_start(w['cvip'][:], cvip_d.ap()[li])
            w['ipz'] = wts.tile([DQ, 2, DQ], F16, tag="ipz", name="ipz", bufs=2)
            nc.sync.dma_start(w['ipz'][:], ipz_d.ap()[li])
            w['cv_b'] = wts.tile([DQ, 2], F32, tag="cv_bb", name="cv_b", bufs=2)
            nc.sync.dma_start(w['cv_b'][:], cvb_d.ap()[li])
            w['xpT'] = wts.tile([DQ, 44], F16, tag="xpT", name="xpT", bufs=2)
            nc.sync.dma_start(w['xpT'][:], xpT_d.ap()[li])
            w['dtT'] = wts.tile([DT_RANK, 3, 128], F16, tag="dtT", name="dtT", bufs=2)
            nc.sync.dma_start(w['dtT'][:], dtT_d.ap()[li])
            w['dt_b'] = wts.tile([128, 3], F32, tag="dt_b", name="dt_b", bufs=2)
            nc.sync.dma_start(w['dt_b'][:], dtb_d.ap()[li])
            w['Apk'] = wts.tile([128, 4], F32, tag="Apk", name="Apk", bufs=2)
            nc.sync.dma_start(w['Apk'][:], apk_d.ap()[li])
            w['Epk'] = wts.tile([128, 1], F32, tag="Epk", name="Epk", bufs=2)
            nc.sync.dma_start(w['Epk'][:], epk_d.ap()[li])
            w['dtT9'] = wts.tile([DT_RANK, DQ], F16, tag="dtT9", name="dtT9", bufs=2)
            nc.sync.dma_start(w['dtT9'][:], dtT9_d.ap()[li])
            w['dtb9'] = wts.tile([DQ, 1], F32, tag="dtb9", name="dtb9", bufs=2)
            nc.sync.dma_start(w['dtb9'][:], dtb9_d.ap()[li])
            w['Dssm'] = wts.tile([DQ, 1], F32, tag="Dssm", name="Dssm", bufs=2)
            nc.sync.dma_start(w['Dssm'][:], Dsm_d.ap()[li])
            w['owT'] = wts.tile([DQ, 2, DQ], F16, tag="owT", name="owT", bufs=2)
            nc.sync.dma_start(w['owT'][:], owT_d.ap()[li])
            w['nrm'] = nrmp.tile([DQ, 2, 3 + n_tok], F16, tag="nrm", name="nrm")
            nc.gpsimd.memset(w['nrm'][:, :, 0:3], 0.0)
            return w

        def all_reduce(src_sb, shape, tag):
            # src_sb: fp16 SBUF tile -> DRAM -> AllReduce(add) over the
            # 4-core batch group -> returns the reduced DRAM tile.
            d_src = dram.tile(shape, F16, tag=f"{tag}s")
            nc.sync.dma_start(d_src[:], src_sb)
            d_dst = dram.tile(shape, F16, tag=f"{tag}d")
            if sim_mode:
                nc.sync.dma_start(d_dst[:], d_src[:])
            else:
                nc.gpsimd.collective_compute(
                    "AllReduce", OP.add, replica_groups=GROUPS,
                    ins=[d_src[:].opt()], outs=[d_dst[:].opt()])
            return d_dst

        def pre_mm(li, ci, w):
            c0, c1 = chunks[ci]
            cw = c1 - c0
            s_cur = s_tiles[li % 2]
            nrm = w['nrm']
            # ---- lp matmuls (shifted taps; bias folded into Act bias) ----
            ps_lp = []
            for m in range(2):
                ps = pa.tile([DQ, TC], F32, tag="mm", name=f"lp{m}")
                first = True
                for kh in range(2):
                    for tap in range(2):
                        nc.tensor.matmul(
                            ps[:, 0:cw], w['lpT'][:, kh, tap, m, :],
                            s_cur[:, kh, c0 + 1 - tap:c0 + 1 - tap + cw],
                            start=first, stop=(kh == 1 and tap == 1))
                        first = False
                ps_lp.append(ps)
            # ---- rmsnorm ----
            p2 = ck1.tile([DQ, 2, TC], F16, tag="p2")
            projsb = ck2.tile([DQ, 2, TC], F16, tag="pj")
            for m in range(2):
                nc.scalar.activation(p2[:, m, 0:cw], ps_lp[m][:, 0:cw],
                                     AF.Square, bias=w['lpb'][:, m:m + 1])
                nc.scalar.activation(projsb[:, m, 0:cw], ps_lp[m][:, 0:cw],
                                     AF.Identity, bias=w['lpb'][:, m:m + 1])
            sq = pa.tile([1, TC], F32, tag="mm", name="sq")
            for m in range(2):
                nc.tensor.matmul(sq[:, 0:cw], ones_ch[:], p2[:, m, 0:cw],
                                 start=(m == 0), stop=(m == 1))
            rstd = ck1.tile([1, TC], F16, tag="rstd", bufs=1)
            nc.scalar.activation(rstd[:, 0:cw], sq[:, 0:cw], AF.Ln,
                                 bias=epsc[:], scale=1.0 / D_MODEL)
            inv16 = ck1.tile([1, TC], F16, tag="inv", bufs=1)
            nc.scalar.activation(inv16[:, 0:cw], rstd[:, 0:cw], AF.Exp,
                                 scale=-0.5)
            ib = pa.tile([DQ, TC], F32, tag="mm", name="ibc")
            nc.tensor.matmul(ib[:, 0:cw], ones_r[:], inv16[:, 0:cw],
                             start=True, stop=True)
            # nrm into the layer-wide halo tile (DVE, PSUM-direct ib)
            ibv = ib[:, 0:cw][:, None]
            _ap = ibv.ap
            _ap[1] = [0, 2]
            ibv.ap = _ap
            nc.vector.tensor_mul(nrm[:, :, 3 + c0:3 + c0 + cw],
                                 projsb[:, :, 0:cw], ibv)
            # ---- own-quarter fused in_proj+conv (8 taps) + all-Act silu ----
            hp_ctx = tc.high_priority()
            hp_ctx.__enter__()
            ps_cv = pa.tile([DQ, TC], F32, tag="mm", name="cv")
            first = True
            for kh in range(2):
                for k in range(D_CONV):
                    nc.tensor.matmul(
                        ps_cv[:, 0:cw], w['cvip'][:, kh, k, :],
                        nrm[:, kh, c0 + k:c0 + k + cw],
                        start=first, stop=(kh == 1 and k == D_CONV - 1))
                    first = False
            ec = ck1.tile([DQ, TC], F16, tag="ec", bufs=2)
            nc.scalar.activation(ec[:, 0:cw], ps_cv[:, 0:cw], AF.Exp,
                                 scale=-1.0, bias=w['cv_b'][:, 1:2])
            sp = ck1.tile([DQ, TC], F16, tag="sp", bufs=2)
            nc.scalar.activation(sp[:, 0:cw], ec[:, 0:cw], AF.Ln, bias=1.0)
            sg = ck1.tile([DQ, TC], F16, tag="sg", bufs=2)
            nc.scalar.activation(sg[:, 0:cw], sp[:, 0:cw], AF.Exp, scale=-1.0)
            # xc = (ps_cv + cv_b) * sigmoid  (Act bias-copy + Pool mul)
            xb = ck1.tile([DQ, TC], F16, tag="xb", bufs=2)
            nc.scalar.activation(xb[:, 0:cw], ps_cv[:, 0:cw], AF.Identity,
                                 bias=w['cv_b'][:, 0:1])
            uq = ck2.tile([DQ, TC], F16, tag="uq", bufs=5)
            nc.gpsimd.tensor_mul(uq[:, 0:cw], xb[:, 0:cw], sg[:, 0:cw])
            # pack u into (si,dd)-partition layout via DRAM staging
            u_d = dram.tile([DQ, TC], F16, tag="ud")
            nc.sync.dma_start(u_d[:, 0:cw], uq[:, 0:cw])
            u_p = big.tile([128, 3, TC], F16, tag="up", bufs=5, name="u_p")
            for si in range(4):
                psrc = u_d[0:32, 0:cw]
                pap = psrc.ap
                pap.insert(1, [32 * TC, 3])
                psrc.ap = pap
                pdst = u_p[32 * si:32 * si + 32, :, 0:cw]
                nc.sync.dma_start(pdst, psrc)
            # ---- z quarter + silu ----
            psz = pa.tile([DQ, TC], F32, tag="mm", name="z")
            for kh in range(2):
                nc.tensor.matmul(psz[:, 0:cw], w['ipz'][:, kh, :],
                                 nrm[:, kh, 3 + c0:3 + c0 + cw],
                                 start=(kh == 0), stop=(kh == 1))
            ez = ck1.tile([DQ, TC], F16, tag="ez", bufs=2)
            nc.scalar.activation(ez[:, 0:cw], psz[:, 0:cw], AF.Exp,
                                 scale=-1.0)
            zsp = ck1.tile([DQ, TC], F16, tag="zsp", bufs=2)
            nc.scalar.activation(zsp[:, 0:cw], ez[:, 0:cw], AF.Ln, bias=1.0)
            zsg = ck1.tile([DQ, TC], F16, tag="zsg", bufs=2)
            nc.scalar.activation(zsg[:, 0:cw], zsp[:, 0:cw], AF.Exp,
                                 scale=-1.0)
            zv = ck1.tile([DQ, TC], F16, tag="zv", bufs=2)
            nc.scalar.activation(zv[:, 0:cw], psz[:, 0:cw], AF.Copy)
            sz = ck2.tile([DQ, TC], F16, tag="sz", bufs=5)
            nc.gpsimd.tensor_mul(sz[:, 0:cw], zv[:, 0:cw], zsg[:, 0:cw])
            # ---- own-quarter xproj partial + AllReduce ----
            ps44 = pb.tile([44, TC], F32, tag="mm2")
            nc.tensor.matmul(ps44[0:44, 0:cw], w['xpT'][:], uq[:, 0:cw],
                             start=True, stop=True)
            xr16 = ck2.tile([44, TC], F16, tag="xr16", bufs=2)
            nc.scalar.activation(xr16[:, 0:cw], ps44[:, 0:cw], AF.Copy)
            xr_d = all_reduce(xr16[:, 0:cw], [44, TC], "xr")
            hp_ctx.__exit__(None, None, None)
            return dict(cw=cw, c0=c0, c1=c1, w=w, uq=uq, sz=sz,
                        xr_d=xr_d, u_p=u_p)

        def pre_bc1(li, ci, p):
            hp_ctx = tc.high_priority()
            hp_ctx.__enter__()
            cw, c0 = p['cw'], p['c0']
            w = p['w']
            xr_d = p['xr_d']
            # dtr rows back to SBUF
            dtr = ck2.tile([DT_RANK, TC], F16, tag="dtr", bufs=3)
            nc.sync.dma_start(dtr[:, 0:cw], xr_d[0:DT_RANK, 0:cw])
            # B/C packed tiles [128=(4si x 32dd), 4sg, T] via one DMA each
            # packed B/C: per-si direct DRAM->SBUF (contiguous dst
            # partitions), B on SP queue, C on Act queue
            BC_pk = big.tile([128, 8, TC], F16, tag="BCpk", bufs=4,
                             name="BC_pk")
            for si in range(4):
                srcv = xr_d[DT_RANK + si:DT_RANK + si + 1, 0:cw]
                sap = srcv.ap
                sap[0] = [0, 32]
                sap.insert(1, [4 * TC, 8])
                srcv.ap = sap
                dstv = BC_pk[32 * si:32 * si + 32, :, 0:cw]
                eng = nc.sync if si % 2 == 0 else nc.scalar
                eng.dma_start(dstv, srcv)
            B_pk = BC_pk[:, 0:4, :]
            C_pk = BC_pk[:, 4:8, :]
            # packed dt chain: 3 j-tiles of [128, T]
            dts = []
            for j in range(3):
                psd = pa.tile([128, TC], F32, tag="mmD", name="dt", bufs=2)
                nc.tensor.matmul(psd[:, 0:cw], w['dtT'][:, j, :],
                                 dtr[:, 0:cw], start=True, stop=True)
                edt = ck1.tile([128, TC], F16, tag="edt", bufs=2)
                nc.scalar.activation(edt[:, 0:cw], psd[:, 0:cw], AF.Exp,
                                     bias=w['dt_b'][:, j:j + 1])
                dt = ck2.tile([128, TC], F16, tag="dt", bufs=8)
                nc.scalar.activation(dt[:, 0:cw], edt[:, 0:cw], AF.Ln,
                                     bias=1.0)
                dts.append(dt)
            # dA bases (sg=0, per-partition scale) + E4 ratio tiles
            dA = [[None] * 3 for _ in range(4)]
            E4 = []
            for j in range(3):
                t = scn.tile([128, TC], F16, tag="dAb", bufs=18,
                             name=f"dAb{j}")
                nc.scalar.activation(t[:, 0:cw], dts[j][:, 0:cw], AF.Exp,
                                     scale=w['Apk'][:, 0:1])
                dA[0][j] = t
                e = scn.tile([128, TC], F16, tag="dAb", bufs=18,
                             name=f"E4{j}")
                nc.scalar.activation(e[:, 0:cw], dts[j][:, 0:cw], AF.Exp,
                                     scale=w['Epk'][:])
                E4.append(e)
            p.update(dA=dA, E4=E4, dts=dts, B_pk=B_pk, C_pk=C_pk)
            hp_ctx.__exit__(None, None, None)

        def pre_bc2(li, ci, p):
            cw = p['cw']
            dA, E4 = p['dA'], p['E4']
            with tc.high_priority():
                for sg in range(1, 4):
                    for j in range(3):
                        t = scn.tile([128, TC], F16, tag="dAm", bufs=18,
                                     name=f"dA{sg}_{j}")
                        eng = nc.gpsimd if (sg, j) in ((1, 0), (2, 1)) else nc.vector
                        eng.tensor_mul(t[:, 0:cw], dA[sg - 1][j][:, 0:cw],
                                       E4[j][:, 0:cw])
                        dA[sg][j] = t

        def pre_dve_b(li, ci, p):
            cw = p['cw']
            with tc.high_priority():
                dtu_p = big.tile([128, 3, TC], F16, tag="dtup", bufs=2,
                                 name="dtu_p")
                for j in range(3):
                    nc.vector.tensor_mul(dtu_p[:, j, 0:cw],
                                         p['dts'][j][:, 0:cw],
                                         p['u_p'][:, j, 0:cw])
                dBuM = [[None] * 3 for _ in range(4)]
                for sg in range(4):
                    for j in range(3):
                        t = big.tile([128, TC], F16, tag="dBuM", bufs=24,
                                     name=f"dBuM{sg}_{j}")
                        nc.vector.tensor_mul(t[:, 0:cw],
                                             dtu_p[:, j, 0:cw],
                                             p['B_pk'][:, sg, 0:cw])
                        dBuM[sg][j] = t
            p.update(dBuM=dBuM)

        def scans(li, ci, p, hp):
            cw = p['cw']
            with tc.high_priority():
                H = [[None] * 3 for _ in range(4)]
                for sg in range(4):
                    for j in range(3):
                        t = big.tile([128, TC], F16, tag="H", bufs=24,
                                     name=f"H{sg}_{j}")
                        init = (0.0 if ci == 0
                                else hp[sg][j][:, p['cw'] - 1:p['cw']])
                        nc.vector.tensor_tensor_scan(
                            t[:, 0:cw], p['dA'][sg][j][:, 0:cw],
                            p['dBuM'][sg][j][:, 0:cw], init,
                            OP.mult, OP.add)
                        H[sg][j] = t
                p['H'] = H
            return H

        def post1(li, ci, p):
            cw = p['cw']
            w = p['w']
            H, C_pk, uq, sz = p['H'], p['C_pk'], p['uq'], p['sz']
            dBuM = p['dBuM']
            with tc.high_priority():
                psy = pa.tile([DQ, TC], F32, tag="mmO", name="psy", bufs=2)
                for j in range(3):
                    for sg in range(4):
                        hc = dBuM[sg][j]
                        heng = nc.gpsimd if sg == 0 else nc.vector
                        heng.tensor_mul(hc[:, 0:cw], H[sg][j][:, 0:cw],
                                        C_pk[:, sg, 0:cw])
                        nc.tensor.matmul(psy[32 * j:32 * j + 32, 0:cw],
                                         red32[:], hc[:, 0:cw],
                                         start=(sg == 0), stop=(sg == 3))
                yD = ck1.tile([DQ, TC], F16, tag="yD", bufs=1)
                nc.vector.scalar_tensor_tensor(yD[:, 0:cw], uq[:, 0:cw],
                                               w['Dssm'][:], psy[:, 0:cw],
                                               OP.mult, OP.add)
            yq = ck1.tile([DQ, TC], F16, tag="yq", bufs=1)
            nc.gpsimd.tensor_mul(yq[:, 0:cw], yD[:, 0:cw], sz[:, 0:cw])
            # own-quarter output projection partial + AllReduce
            po = ck1.tile([DQ, 2, TC], F16, tag="po", bufs=2)
            for m in range(2):
                ps = pa.tile([DQ, TC], F32, tag="mmO", name=f"out{m}", bufs=2)
                nc.tensor.matmul(ps[:, 0:cw], w['owT'][:, m, :],
                                 yq[:, 0:cw], start=True, stop=True)
                nc.scalar.activation(po[:, m, 0:cw], ps[:, 0:cw], AF.Copy)
            p['or_d'] = all_reduce(po[:, :, 0:cw], [DQ, 2, TC], "or")

        def post2(li, ci, p):
            cw, c0 = p['cw'], p['c0']
            s_cur = s_tiles[li % 2]
            s_nxt = s_tiles[(li + 1) % 2]
            red = ck1.tile([DQ, 2, TC], F16, tag="red", bufs=2)
            nc.sync.dma_start(red[:, :, 0:cw], p['or_d'][:])
            nc.vector.tensor_add(s_nxt[:, :, 1 + c0:1 + c0 + cw],
                                 red[:, :, 0:cw],
                                 s_cur[:, :, 1 + c0:1 + c0 + cw])
            if li == depth - 1:
                fp2 = ck1.tile([DQ, 2, TC], F16, tag="p2", name="fp2")
                nc.scalar.activation(fp2[:, :, 0:cw],
                                     s_nxt[:, :, 1 + c0:1 + c0 + cw],
                                     AF.Square)
                fsq = pa.tile([1, TC], F32, tag="mm", name="fsq")
                for m in range(2):
                    nc.tensor.matmul(fsq[:, 0:cw], ones_ch[:],
                                     fp2[:, m, 0:cw],
                                     start=(m == 0), stop=(m == 1))
                frs = ck1.tile([1, TC], F16, tag="rstd", name="frs", bufs=1)
                nc.scalar.activation(frs[:, 0:cw], fsq[:, 0:cw], AF.Ln,
                                     bias=epsc[:], scale=1.0 / D_MODEL)
                finv = ck1.tile([1, TC], F16, tag="inv", name="finv", bufs=1)
                nc.scalar.activation(finv[:, 0:cw], frs[:, 0:cw], AF.Exp,
                                     scale=-0.5)
                fib = pa.tile([DQ, TC], F32, tag="mm", name="fib")
                nc.tensor.matmul(fib[:, 0:cw], ones_r[:], finv[:, 0:cw],
                                 start=True, stop=True)
                for m in range(2):
                    fn = ck1.tile([DQ, TC], F16, tag="fn", name=f"fn{m}",
                                  bufs=2)
                    nc.vector.tensor_mul(fn[:, 0:cw],
                                         s_nxt[:, m, 1 + c0:1 + c0 + cw],
                                         fib[:, 0:cw])
                    fo = ck1.tile([DQ, TC], F16, tag="fo", name=f"fo{m}",
                                  bufs=2)
                    nc.vector.tensor_scalar_mul(fo[:, 0:cw], fn[:, 0:cw],
                                                nfw[:, m:m + 1])
                    nc.sync.dma_start(out_d.ap()[m, :, c0:c0 + cw],
                                      fo[:, 0:cw])

        # -------- flat software-pipelined emission (depth-3) --------
        items = [(li, ci) for li in range(depth) for ci in range(NCH)]
        emitted_w = {}

        def w_for(li):
            if li not in emitted_w:
                emitted_w[li] = load_weights(li)
            return emitted_w[li]

        P = {}
        pend = None            # awaiting POST1
        pend2 = None
        pend3 = None           # awaiting POST2 (lag 3)
        hprev = None
        P[items[0]] = pre_mm(*items[0], w_for(items[0][0]))
        P[items[1]] = pre_mm(*items[1], w_for(items[1][0]))
        P[items[2]] = pre_mm(*items[2], w_for(items[2][0]))
        pre_bc1(*items[0], P[items[0]])
        pre_bc1(*items[1], P[items[1]])
        for i, (li, ci) in enumerate(items):
            p = P.pop((li, ci))
            pre_bc2(li, ci, p)
            if pend is not None:
                post1(*pend)
            if pend3 is not None:
                post2(*pend3)
            if i + 3 < len(items):
                P[items[i + 3]] = pre_mm(*items[i + 3],
                                         w_for(items[i + 3][0]))
            pre_dve_b(li, ci, p)
            hprev = scans(li, ci, p, hprev)
            if i + 2 < len(items):
                pre_bc1(*items[i + 2], P[items[i + 2]])
            pend3 = pend2
            pend2 = pend
            pend = (li, ci, p)
        post1(*pend)
        if pend3 is not None:
            post2(*pend3)
        post2(*pend2)
        post2(*pend)

    nc.compile()
    return nc


def _prep_inputs(inputs, depth=DEPTH):
    f = lambda k: np.asarray(inputs[k], np.float32)
    x = f("x")
    B = x.shape[0]
    lp_w, lp_b = f("lp_w"), f("lp_b")
    norm_w = f("norm_w")
    ipw = f("in_proj_w")
    conv_w, conv_b = f("conv_w"), f("conv_b")
    xpw = f("xproj_w")
    dt_w, dt_b = f("dt_w"), f("dt_b")
    A_log, D_ssm = f("A_log"), f("D_ssm")
    out_w = f("out_w")
    nfw = f("normf_w")
    proj_w, proj_b = f("proj_w"), f("proj_b")

    A_vals = -np.exp(A_log[:, 0, :]).astype(np.float32)

    h = np.einsum("bchw,dc->bdhw", x, proj_w) + proj_b[None, :, None, None]
    n_tok = x.shape[2] * x.shape[3]
    s0 = h.reshape(B, D_MODEL, n_tok).astype(np.float32)

    Wip = ipw * norm_w[:, None, :]

    W1 = lp_w[:, :, :D_MODEL]
    W2 = lp_w[:, :, D_MODEL:]
    W1p = W1 + W2
    W2p = -W2
    lpT = np.zeros((depth, DQ, 2, 2, 2, DQ), np.float32)
    for kh in range(2):
        for m in range(2):
            blk1 = W1p[:, m * DQ:(m + 1) * DQ, kh * DQ:(kh + 1) * DQ]
            blk2 = W2p[:, m * DQ:(m + 1) * DQ, kh * DQ:(kh + 1) * DQ]
            lpT[:, :, kh, 0, m, :] = blk1.transpose(0, 2, 1)
            lpT[:, :, kh, 1, m, :] = blk2.transpose(0, 2, 1)
    lpb = np.ascontiguousarray(lp_b.reshape(depth, 2, DQ).transpose(0, 2, 1))
    nfw2 = np.ascontiguousarray(nfw.reshape(2, DQ).T)

    in_maps = []
    for core in range(NCORES):
        b, q = core // 4, core % 4
        qsl = slice(q * DQ, (q + 1) * DQ)

        # own-quarter fused in_proj+conv weights:
        # cvip[li, e(part), kh, k, dcol] = Wip[li, q*96+dcol, kh*96+e]
        #                                  * conv_w[li, q*96+dcol, k]
        cvip = np.zeros((depth, DQ, 2, D_CONV, DQ), np.float32)
        for kh in range(2):
            wb = Wip[:, qsl, kh * DQ:(kh + 1) * DQ]     # (depth, d, e)
            for k in range(D_CONV):
                cvip[:, :, kh, k, :] = (
                    wb * conv_w[:, qsl, k][:, :, None]
                ).transpose(0, 2, 1)
        ipz = np.zeros((depth, DQ, 2, DQ), np.float32)
        for kh in range(2):
            ipz[:, :, kh, :] = Wip[:, D_INNER + q * DQ:D_INNER + (q + 1) * DQ,
                                   kh * DQ:(kh + 1) * DQ].transpose(0, 2, 1)
        cvb_own = conv_b[:, qsl]
        cvb = np.stack([cvb_own, -cvb_own], 2).astype(np.float32)
        xpT = np.ascontiguousarray(
            xpw[:, :, qsl].transpose(0, 2, 1))           # (depth, DQ, 44)
        dtw_q = dt_w[:, qsl, :]                       # (depth, 96, 12)
        dtb_q = dt_b[:, qsl]                          # (depth, 96)
        p_arr = np.arange(128)
        si_p, dd_p = p_arr // 32, p_arr % 32
        # packed dt matmul weights: dtT[li, r, j, p] = dtw_q[li, 32j+dd(p), r]
        dtT = np.zeros((depth, DT_RANK, 3, 128), np.float32)
        dtb_pk = np.zeros((depth, 128, 3), np.float32)
        for j in range(3):
            dtT[:, :, j, :] = dtw_q[:, 32 * j + dd_p, :].transpose(0, 2, 1)
            dtb_pk[:, :, j] = dtb_q[:, 32 * j + dd_p]
        Apk = np.zeros((depth, 128, 4), np.float32)
        for sg in range(4):
            Apk[:, :, sg] = A_vals[:, 4 * sg + si_p]
        Epk = (A_vals[:, si_p + 4] - A_vals[:, si_p])[:, :, None]
        red32 = (dd_p[:, None] == np.arange(32)[None, :]).astype(np.float16)
        dtT9 = np.ascontiguousarray(dtw_q.transpose(0, 2, 1))
        # own-quarter out projection: owT[li, dq(contract), m, e]
        #   = out_w[li, m*96+e, q*96+dq]
        owT = np.ascontiguousarray(
            out_w[:, :, qsl].transpose(0, 2, 1).reshape(depth, DQ, 2, DQ))

        in_maps.append({
            "s0": np.ascontiguousarray(
                s0[b].reshape(2, DQ, n_tok).transpose(1, 0, 2)
            ).astype(np.float16),
            "lpT": lpT.astype(np.float16),
            "lpb": lpb,
            "cvip": np.ascontiguousarray(cvip).astype(np.float16),
            "ipz": np.ascontiguousarray(ipz).astype(np.float16),
            "cv_b": np.ascontiguousarray(cvb),
            "xpT": xpT.astype(np.float16),
            "dtT": np.ascontiguousarray(dtT).astype(np.float16),
            "dt_b": np.ascontiguousarray(dtb_pk),
            "Apk": np.ascontiguousarray(Apk),
            "Epk": np.ascontiguousarray(Epk.astype(np.float32)),
            "red32": np.ascontiguousarray(red32),
            "dtT9": dtT9.astype(np.float16),
            "dtb9": np.ascontiguousarray(dtb_q[:, :, None].astype(np.float32)),
            "Dssm": np.ascontiguousarray(D_ssm[:, qsl, None]),
            "owT": owT.astype(np.float16),
            "nfw": nfw2,
        })
    return in_maps, A_vals, x.shape


def kernel(**inputs):
    in_maps, A_vals, xshape = _prep_inputs(inputs)
    key = ("v5", A_vals.tobytes())
    if key not in _CACHE:
        _CACHE[key] = _build(A_vals)
    nc = _CACHE[key]
    try:
        res = run_bass_kernel_spmd(nc, in_maps, core_ids=list(range(NCORES)))
    except Exception:
        res = run_bass_kernel_spmd(nc, in_maps, core_ids=list(range(NCORES)))
    B, _, H, W = xshape
    out = np.zeros((B, D_MODEL, H * W), np.float32)
    for b in range(B):
        r = res.results[b * 4]["out_s"]
        out[b, :DQ] = np.float32(r[0])
        out[b, DQ:] = np.float32(r[1])
    return out.reshape(B, D_MODEL, H, W)


# revision 34
# speedup vs baseline: 1.0235x; 1.0235x over previous
"""Trainium2 Bass kernel for nn_DiVimEncoder (Vision-Mamba encoder), v4.

Sharding: 8 cores = batch(2) x d_inner-quarter(4). Each core runs the full
token stream feature-major; each core owns a 96-channel quarter of
d_inner. The d_model-wide lp/rmsnorm chain is replicated inside a batch
group; conv+in_proj, xproj and the output projection contract only the
core's own quarter, with fp16 AllReduce collectives combining the
quarter-partial xproj (44 x T) and output (192 x T) products.

v4 structure:
  - all matmul operands fp16 (1 cycle/row on PE)
  - software-pipelined emission per chunk: PRE_MM(c+1) [matmuls + silu]
    -> PRE_BC(c) [consume xproj AllReduce: dt/ladder/B/C] -> POST1(c-1)
    -> POST2(c-2) -> dBuM(c) -> SCANS(c); collectives get a full step
    of latency hiding
  - depthwise conv folded into in_proj as shifted matmul taps (8-tap
    accumulation for the core's own quarter only)
  - silu entirely on Act (exp/ln/exp sigmoid) + one Pool STT mul reading
    PSUM directly; lp/conv biases folded into Act bias APs
  - residual stream s and nrm kept in SBUF across layers
  - scan tree-reduce + HC on DVE; dA ladder split Act/DVE/Pool
"""
import numpy as np
from contextlib import ExitStack

import concourse.bass as bass
import concourse.bacc as bacc
import concourse.tile as tile
import concourse.mybir as mybir
from concourse.bass_utils import run_bass_kernel_spmd

F32 = mybir.dt.float32
F16 = mybir.dt.float16
AF = mybir.ActivationFunctionType
OP = mybir.AluOpType

D_MODEL = 192
DEPTH = 12
D_INNER = 384
DS = 16
D_CONV = 4
DT_RANK = 12
EPS = 1e-5
N = 2304
DQ = 96
TC = 384
NCH = N // TC
NCORES = 8
GROUPS = [[0, 1, 2, 3], [4, 5, 6, 7]]

LAD_EXP_S = [0, 1, 2, 3, 7]
LAD_MUL_S = [(4, 0, 3), (5, 1, 3), (6, 2, 3), (8, 0, 7), (9, 1, 7),
             (10, 2, 7), (11, 3, 7), (12, 4, 7), (13, 5, 7), (14, 6, 7),
             (15, 7, 7)]

_CACHE = {}

_gat_patched = False


def _patch_act_tables():
    global _gat_patched
    if _gat_patched:
        return
    from concourse import hw_specs
    real = hw_specs.get_activation_tables

    def patched(arch):
        t = dict(real(arch))
        keep_name = "natural_log_exp_and_others"
        keep = t[keep_name]
        return {name: (funcs if name == keep_name else funcs - keep)
                for name, funcs in t.items()}

    bacc.get_activation_tables = patched
    _gat_patched = True


def _build(A_vals, depth=DEPTH, n_tok=N, sim_mode=False):
    _patch_act_tables()
    chunks = [(c, min(c + TC, n_tok)) for c in range(0, n_tok, TC)]
    nc = bacc.Bacc("TRN2", target_bir_lowering=False, debug=False,
                   enable_asserts=True, num_devices=NCORES)

    s0_d = nc.dram_tensor("s0", [DQ, 2, n_tok], F16, kind="ExternalInput")
    lpT_d = nc.dram_tensor("lpT", [depth, DQ, 2, 2, 2, DQ], F16, kind="ExternalInput")
    lpb_d = nc.dram_tensor("lpb", [depth, DQ, 2], F32, kind="ExternalInput")
    cvip_d = nc.dram_tensor("cvip", [depth, DQ, 2, D_CONV, DQ], F16, kind="ExternalInput")
    ipz_d = nc.dram_tensor("ipz", [depth, DQ, 2, DQ], F16, kind="ExternalInput")
    cvb_d = nc.dram_tensor("cv_b", [depth, DQ, 2], F32, kind="ExternalInput")
    xpT_d = nc.dram_tensor("xpT", [depth, DQ, 44], F16, kind="ExternalInput")
    dtT_d = nc.dram_tensor("dtT", [depth, DT_RANK, 3, 128], F16, kind="ExternalInput")
    dtb_d = nc.dram_tensor("dt_b", [depth, 128, 3], F32, kind="ExternalInput")
    apk_d = nc.dram_tensor("Apk", [depth, 128, 4], F32, kind="ExternalInput")
    epk_d = nc.dram_tensor("Epk", [depth, 128, 1], F32, kind="ExternalInput")
    r32_d = nc.dram_tensor("red32", [128, 32], F16, kind="ExternalInput")
    dtT9_d = nc.dram_tensor("dtT9", [depth, DT_RANK, DQ], F16, kind="ExternalInput")
    dtb9_d = nc.dram_tensor("dtb9", [depth, DQ, 1], F32, kind="ExternalInput")
    Dsm_d = nc.dram_tensor("Dssm", [depth, DQ, 1], F32, kind="ExternalInput")
    owT_d = nc.dram_tensor("owT", [depth, DQ, 2, DQ], F16, kind="ExternalInput")
    nfw_d = nc.dram_tensor("nfw", [DQ, 2], F32, kind="ExternalInput")
    out_d = nc.dram_tensor("out_s", [2, DQ, n_tok], F16, kind="ExternalOutput")

    with tile.TileContext(nc) as tc, ExitStack() as ctx:
        consts = ctx.enter_context(tc.tile_pool(name="consts", bufs=1))
        sres = ctx.enter_context(tc.tile_pool(name="sres", bufs=1))
        nrmp = ctx.enter_context(tc.tile_pool(name="nrmp", bufs=2))
        wts = ctx.enter_context(tc.tile_pool(name="wts", bufs=1))
        ck2 = ctx.enter_context(tc.tile_pool(name="ck2", bufs=2))
        ck1 = ctx.enter_context(tc.tile_pool(name="ck1", bufs=1))
        scn = ctx.enter_context(tc.tile_pool(name="scn", bufs=18))
        big = ctx.enter_context(tc.tile_pool(name="big", bufs=2))
        pa = ctx.enter_context(tc.tile_pool(name="pa", bufs=3, space="PSUM"))
        pb = ctx.enter_context(tc.tile_pool(name="pb", bufs=1, space="PSUM"))
        dram = ctx.enter_context(tc.tile_pool(name="dram", bufs=3, space="DRAM"))

        ones_r = consts.tile([1, DQ], F16)
        nc.gpsimd.memset(ones_r[:], 1.0)
        ones_ch = consts.tile([DQ, 1], F16)
        nc.gpsimd.memset(ones_ch[:], 1.0)
        nfw = consts.tile([DQ, 2], F32)
        nc.sync.dma_start(nfw[:], nfw_d.ap())
        epsc = consts.tile([1, 1], F32)
        nc.gpsimd.memset(epsc[:], EPS)
        red32 = consts.tile([128, 32], F16)
        nc.sync.dma_start(red32[:], r32_d.ap())

        s_tiles = []
        for pi in range(2):
            st = sres.tile([DQ, 2, 1 + n_tok], F16, tag=f"s{pi}", name=f"s{pi}")
            nc.gpsimd.memset(st[:, :, 0:1], 0.0)
            s_tiles.append(st)
        nc.sync.dma_start(s_tiles[0][:, :, 1:1 + n_tok], s0_d.ap())

        W = {}
        P = {}

        def load_weights(li):
            w = {}
            w['lpT'] = wts.tile([DQ, 2, 2, 2, DQ], F16, tag="lpT", name="lpT", bufs=2)
            nc.sync.dma_start(w['lpT'][:], lpT_d.ap()[li])
            w['lpb'] = wts.tile([DQ, 2], F32, tag="lpb", name="lpb", bufs=2)
            nc.sync.dma_start(w['lpb'][:], lpb_d.ap()[li])
            w['cvip'] = wts.tile([DQ, 2, D_CONV, DQ], F16, tag="cvip", name="cvip", bufs=2)
            nc.sync.dma_start(w['cvip'][:], cvip_d.ap()[li])
            w['ipz'] = wts.tile([DQ, 2, DQ], F16, tag="ipz", name="ipz", bufs=2)
            nc.sync.dma_start(w['ipz'][:], ipz_d.ap()[li])
            w['cv_b'] = wts.tile([DQ, 2], F32, tag="cv_bb", name="cv_b", bufs=2)
            nc.sync.dma_start(w['cv_b'][:], cvb_d.ap()[li])
            w['xpT'] = wts.tile([DQ, 44], F16, tag="xpT", name="xpT", bufs=2)
            nc.sync.dma_start(w['xpT'][:], xpT_d.ap()[li])
            w['dtT'] = wts.tile([DT_RANK, 3, 128], F16, tag="dtT", name="dtT", bufs=2)
            nc.sync.dma_start(w['dtT'][:], dtT_d.ap()[li])
            w['dt_b'] = wts.tile([128, 3], F32, tag="dt_b", name="dt_b", bufs=2)
            nc.sync.dma_start(w['dt_b'][:], dtb_d.ap()[li])
            w['Apk'] = wts.tile([128, 4], F32, tag="Apk", name="Apk", bufs=2)
            nc.sync.dma_start(w['Apk'][:], apk_d.ap()[li])
            w['Epk'] = wts.tile([128, 1], F32, tag="Epk", name="Epk", bufs=2)
            nc.sync.dma_start(w['Epk'][:], epk_d.ap()[li])
            w['dtT9'] = wts.tile([DT_RANK, DQ], F16, tag="dtT9", name="dtT9", bufs=2)
            nc.sync.dma_start(w['dtT9'][:], dtT9_d.ap()[li])
            w['dtb9'] = wts.tile([DQ, 1], F32, tag="dtb9", name="dtb9", bufs=2)
            nc.sync.dma_start(w['dtb9'][:], dtb9_d.ap()[li])
            w['Dssm'] = wts.tile([DQ, 1], F32, tag="Dssm", name="Dssm", bufs=2)
            nc.sync.dma_start(w['Dssm'][:], Dsm_d.ap()[li])
            w['owT'] = wts.tile([DQ, 2, DQ], F16, tag="owT", name="owT", bufs=2)
            nc.sync.dma_start(w['owT'][:], owT_d.ap()[li])
            w['nrm'] = nrmp.tile([DQ, 2, 3 + n_tok], F16, tag="nrm", name="nrm")
            nc.gpsimd.memset(w['nrm'][:, :, 0:3], 0.0)
            return w

        def all_reduce(src_sb, shape, tag):
            # src_sb: fp16 SBUF tile -> DRAM -> AllReduce(add) over the
            # 4-core batch group -> returns the reduced DRAM tile.
            d_src = dram.tile(shape, F16, tag=f"{tag}s")
            nc.sync.dma_start(d_src[:], src_sb)
            d_dst = dram.tile(shape, F16, tag=f"{tag}d")
            if sim_mode:
                nc.sync.dma_start(d_dst[:], d_src[:])
            else:
                nc.gpsimd.collective_compute(
                    "AllReduce", OP.add, replica_groups=GROUPS,
                    ins=[d_src[:].opt()], outs=[d_dst[:].opt()])
            return d_dst

        def pre_mm(li, ci, w):
            c0, c1 = chunks[ci]
            cw = c1 - c0
            s_cur = s_tiles[li % 2]
            nrm = w['nrm']
            # ---- lp matmuls (shifted taps; bias folded into Act bias) ----
            ps_lp = []
            for m in range(2):
                ps = pa.tile([DQ, TC], F32, tag="mm", name=f"lp{m}")
                first = True
                for kh in range(2):
                    for tap in range(2):
                        nc.tensor.matmul(
                            ps[:, 0:cw], w['lpT'][:, kh, tap, m, :],
                            s_cur[:, kh, c0 + 1 - tap:c0 + 1 - tap + cw],
                            start=first, stop=(kh == 1 and tap == 1))
                        first = False
                ps_lp.append(ps)
            # ---- rmsnorm ----
            p2 = ck1.tile([DQ, 2, TC], F16, tag="p2")
            projsb = ck2.tile([DQ, 2, TC], F16, tag="pj")
            for m in range(2):
                nc.scalar.activation(p2[:, m, 0:cw], ps_lp[m][:, 0:cw],
                                     AF.Square, bias=w['lpb'][:, m:m + 1])
                nc.scalar.activation(projsb[:, m, 0:cw], ps_lp[m][:, 0:cw],
                                     AF.Identity, bias=w['lpb'][:, m:m + 1])
            sq = pa.tile([1, TC], F32, tag="mm", name="sq")
            for m in range(2):
                nc.tensor.matmul(sq[:, 0:cw], ones_ch[:], p2[:, m, 0:cw],
                                 start=(m == 0), stop=(m == 1))
            rstd = ck1.tile([1, TC], F16, tag="rstd", bufs=1)
            nc.scalar.activation(rstd[:, 0:cw], sq[:, 0:cw], AF.Ln,
                                 bias=epsc[:], scale=1.0 / D_MODEL)
            inv16 = ck1.tile([1, TC], F16, tag="inv", bufs=1)
            nc.scalar.activation(inv16[:, 0:cw], rstd[:, 0:cw], AF.Exp,
                                 scale=-0.5)
            ib = pa.tile([DQ, TC], F32, tag="mm", name="ibc")
            nc.tensor.matmul(ib[:, 0:cw], ones_r[:], inv16[:, 0:cw],
                             start=True, stop=True)
            # nrm into the layer-wide halo tile (DVE, PSUM-direct ib)
            ibv = ib[:, 0:cw][:, None]
            _ap = ibv.ap
            _ap[1] = [0, 2]
            ibv.ap = _ap
            nc.vector.tensor_mul(nrm[:, :, 3 + c0:3 + c0 + cw],
                                 projsb[:, :, 0:cw], ibv)
            # ---- own-quarter fused in_proj+conv (8 taps) + all-Act silu ----
            hp_ctx = tc.high_priority()
            hp_ctx.__enter__()
            ps_cv = pa.tile([DQ, TC], F32, tag="mm", name="cv")
            first = True
            for kh in range(2):
                for k in range(D_CONV):
                    nc.tensor.matmul(
                        ps_cv[:, 0:cw], w['cvip'][:, kh, k, :],
                        nrm[:, kh, c0 + k:c0 + k + cw],
                        start=first, stop=(kh == 1 and k == D_CONV - 1))
                    first = False
            ec = ck1.tile([DQ, TC], F16, tag="ec", bufs=2)
            nc.scalar.activation(ec[:, 0:cw], ps_cv[:, 0:cw], AF.Exp,
                                 scale=-1.0, bias=w['cv_b'][:, 1:2])
            sp = ck1.tile([DQ, TC], F16, tag="sp", bufs=2)
            nc.scalar.activation(sp[:, 0:cw], ec[:, 0:cw], AF.Ln, bias=1.0)
            sg = ck1.tile([DQ, TC], F16, tag="sg", bufs=2)
            nc.scalar.activation(sg[:, 0:cw], sp[:, 0:cw], AF.Exp, scale=-1.0)
            # xc = (ps_cv + cv_b) * sigmoid  (Act bias-copy + Pool mul)
            xb = ck1.tile([DQ, TC], F16, tag="xb", bufs=2)
            nc.scalar.activation(xb[:, 0:cw], ps_cv[:, 0:cw], AF.Identity,
                                 bias=w['cv_b'][:, 0:1])
            uq = ck2.tile([DQ, TC], F16, tag="uq", bufs=6)
            nc.gpsimd.tensor_mul(uq[:, 0:cw], xb[:, 0:cw], sg[:, 0:cw])
            # pack u into (si,dd)-partition layout via DRAM staging
            u_d = dram.tile([DQ, TC], F16, tag="ud")
            nc.sync.dma_start(u_d[:, 0:cw], uq[:, 0:cw])
            u_p = big.tile([128, 3, TC], F16, tag="up", bufs=5, name="u_p")
            for si in range(4):
                psrc = u_d[0:32, 0:cw]
                pap = psrc.ap
                pap.insert(1, [32 * TC, 3])
                psrc.ap = pap
                pdst = u_p[32 * si:32 * si + 32, :, 0:cw]
                nc.sync.dma_start(pdst, psrc)
            # ---- z quarter + silu ----
            psz = pa.tile([DQ, TC], F32, tag="mm", name="z")
            for kh in range(2):
                nc.tensor.matmul(psz[:, 0:cw], w['ipz'][:, kh, :],
                                 nrm[:, kh, 3 + c0:3 + c0 + cw],
                                 start=(kh == 0), stop=(kh == 1))
            ez = ck1.tile([DQ, TC], F16, tag="ez", bufs=2)
            nc.scalar.activation(ez[:, 0:cw], psz[:, 0:cw], AF.Exp,
                                 scale=-1.0)
            zsp = ck1.tile([DQ, TC], F16, tag="zsp", bufs=2)
            nc.scalar.activation(zsp[:, 0:cw], ez[:, 0:cw], AF.Ln, bias=1.0)
            zsg = ck1.tile([DQ, TC], F16, tag="zsg", bufs=2)
            nc.scalar.activation(zsg[:, 0:cw], zsp[:, 0:cw], AF.Exp,
                                 scale=-1.0)
            zv = ck1.tile([DQ, TC], F16, tag="zv", bufs=2)
            nc.scalar.activation(zv[:, 0:cw], psz[:, 0:cw], AF.Copy)
            sz = ck2.tile([DQ, TC], F16, tag="sz", bufs=6)
            nc.gpsimd.tensor_mul(sz[:, 0:cw], zv[:, 0:cw], zsg[:, 0:cw])
            # ---- own-quarter xproj partial + AllReduce ----
            ps44 = pb.tile([44, TC], F32, tag="mm2")
            nc.tensor.matmul(ps44[0:44, 0:cw], w['xpT'][:], uq[:, 0:cw],
                             start=True, stop=True)
            xr16 = ck2.tile([44, TC], F16, tag="xr16", bufs=2)
            nc.scalar.activation(xr16[:, 0:cw], ps44[:, 0:cw], AF.Copy)
            xr_d = all_reduce(xr16[:, 0:cw], [44, TC], "xr")
            hp_ctx.__exit__(None, None, None)
            return dict(cw=cw, c0=c0, c1=c1, w=w, uq=uq, sz=sz,
                        xr_d=xr_d, u_p=u_p)

        def pre_bc1(li, ci, p):
            hp_ctx = tc.high_priority()
            hp_ctx.__enter__()
            cw, c0 = p['cw'], p['c0']
            w = p['w']
            xr_d = p['xr_d']
            # dtr rows back to SBUF
            dtr = ck2.tile([DT_RANK, TC], F16, tag="dtr", bufs=3)
            nc.sync.dma_start(dtr[:, 0:cw], xr_d[0:DT_RANK, 0:cw])
            # B/C packed tiles [128=(4si x 32dd), 4sg, T] via one DMA each
            # packed B/C: per-si direct DRAM->SBUF (contiguous dst
            # partitions), B on SP queue, C on Act queue
            BC_pk = big.tile([128, 8, TC], F16, tag="BCpk", bufs=4,
                             name="BC_pk")
            for si in range(4):
                srcv = xr_d[DT_RANK + si:DT_RANK + si + 1, 0:cw]
                sap = srcv.ap
                sap[0] = [0, 32]
                sap.insert(1, [4 * TC, 8])
                srcv.ap = sap
                dstv = BC_pk[32 * si:32 * si + 32, :, 0:cw]
                eng = nc.sync if si % 2 == 0 else nc.scalar
                eng.dma_start(dstv, srcv)
            B_pk = BC_pk[:, 0:4, :]
            C_pk = BC_pk[:, 4:8, :]
            # packed dt chain: 3 j-tiles of [128, T]
            dts = []
            for j in range(3):
                psd = pa.tile([128, TC], F32, tag="mmD", name="dt", bufs=2)
                nc.tensor.matmul(psd[:, 0:cw], w['dtT'][:, j, :],
                                 dtr[:, 0:cw], start=True, stop=True)
                edt = ck1.tile([128, TC], F16, tag="edt", bufs=6)
                nc.scalar.activation(edt[:, 0:cw], psd[:, 0:cw], AF.Exp,
                                     bias=w['dt_b'][:, j:j + 1])
                dt = ck2.tile([128, TC], F16, tag="dt", bufs=12)
                nc.scalar.activation(dt[:, 0:cw], edt[:, 0:cw], AF.Ln,
                                     bias=1.0)
                dts.append(dt)
            # dA bases (sg=0, per-partition scale) + E4 ratio tiles
            dA = [[None] * 3 for _ in range(4)]
            E4 = []
            for j in range(3):
                t = scn.tile([128, TC], F16, tag="dAb", bufs=18,
                             name=f"dAb{j}")
                nc.scalar.activation(t[:, 0:cw], dts[j][:, 0:cw], AF.Exp,
                                     scale=w['Apk'][:, 0:1])
                dA[0][j] = t
                e = scn.tile([128, TC], F16, tag="dAb", bufs=18,
                             name=f"E4{j}")
                nc.scalar.activation(e[:, 0:cw], dts[j][:, 0:cw], AF.Exp,
                                     scale=w['Epk'][:])
                E4.append(e)
            p.update(dA=dA, E4=E4, dts=dts, B_pk=B_pk, C_pk=C_pk)
            hp_ctx.__exit__(None, None, None)

        def pre_bc2(li, ci, p):
            cw = p['cw']
            dA, E4 = p['dA'], p['E4']
            with tc.high_priority():
                for sg in range(1, 4):
                    for j in range(3):
                        t = scn.tile([128, TC], F16, tag="dAm", bufs=18,
                                     name=f"dA{sg}_{j}")
                        eng = nc.gpsimd if (sg, j) in ((1, 0), (2, 1), (3, 2), (2, 0)) else nc.vector
                        eng.tensor_mul(t[:, 0:cw], dA[sg - 1][j][:, 0:cw],
                                       E4[j][:, 0:cw])
                        dA[sg][j] = t

        def pre_dve_b(li, ci, p):
            cw = p['cw']
            with tc.high_priority():
                dtu_p = big.tile([128, 3, TC], F16, tag="dtup", bufs=2,
                                 name="dtu_p")
                for j in range(3):
                    nc.vector.tensor_mul(dtu_p[:, j, 0:cw],
                                         p['dts'][j][:, 0:cw],
                                         p['u_p'][:, j, 0:cw])
                dBuM = [[None] * 3 for _ in range(4)]
                for sg in range(4):
                    for j in range(3):
                        t = big.tile([128, TC], F16, tag="dBuM", bufs=24,
                                     name=f"dBuM{sg}_{j}")
                        nc.vector.tensor_mul(t[:, 0:cw],
                                             dtu_p[:, j, 0:cw],
                                             p['B_pk'][:, sg, 0:cw])
                        dBuM[sg][j] = t
            p.update(dBuM=dBuM)

        def scans(li, ci, p, hp):
            cw = p['cw']
            with tc.high_priority():
                H = [[None] * 3 for _ in range(4)]
                for sg in range(4):
                    for j in range(3):
                        t = big.tile([128, TC], F16, tag="H", bufs=24,
                                     name=f"H{sg}_{j}")
                        init = (0.0 if ci == 0
                                else hp[sg][j][:, p['cw'] - 1:p['cw']])
                        nc.vector.tensor_tensor_scan(
                            t[:, 0:cw], p['dA'][sg][j][:, 0:cw],
                            p['dBuM'][sg][j][:, 0:cw], init,
                            OP.mult, OP.add)
                        H[sg][j] = t
                p['H'] = H
            return H

        def post1(li, ci, p):
            cw = p['cw']
            w = p['w']
            H, C_pk, uq, sz = p['H'], p['C_pk'], p['uq'], p['sz']
            dBuM = p['dBuM']
            with tc.high_priority():
                psy = pa.tile([DQ, TC], F32, tag="mmO", name="psy", bufs=2)
                for j in range(3):
                    for sg in range(4):
                        hc = dBuM[sg][j]
                        heng = nc.gpsimd if sg == 0 else nc.vector
                        heng.tensor_mul(hc[:, 0:cw], H[sg][j][:, 0:cw],
                                        C_pk[:, sg, 0:cw])
                        nc.tensor.matmul(psy[32 * j:32 * j + 32, 0:cw],
                                         red32[:], hc[:, 0:cw],
                                         start=(sg == 0), stop=(sg == 3))
                yD = ck1.tile([DQ, TC], F16, tag="yD", bufs=1)
                nc.vector.scalar_tensor_tensor(yD[:, 0:cw], uq[:, 0:cw],
                                               w['Dssm'][:], psy[:, 0:cw],
                                               OP.mult, OP.add)
            yq = ck1.tile([DQ, TC], F16, tag="yq", bufs=1)
            nc.gpsimd.tensor_mul(yq[:, 0:cw], yD[:, 0:cw], sz[:, 0:cw])
            # own-quarter output projection partial + AllReduce
            po = ck1.tile([DQ, 2, TC], F16, tag="po", bufs=2)
            for m in range(2):
                ps = pa.tile([DQ, TC], F32, tag="mmO", name=f"out{m}", bufs=2)
                nc.tensor.matmul(ps[:, 0:cw], w['owT'][:, m, :],
                                 yq[:, 0:cw], start=True, stop=True)
                nc.scalar.activation(po[:, m, 0:cw], ps[:, 0:cw], AF.Copy)
            p['or_d'] = all_reduce(po[:, :, 0:cw], [DQ, 2, TC], "or")

        def post2(li, ci, p):
            cw, c0 = p['cw'], p['c0']
            s_cur = s_tiles[li % 2]
            s_nxt = s_tiles[(li + 1) % 2]
            red = ck1.tile([DQ, 2, TC], F16, tag="red", bufs=2)
            nc.sync.dma_start(red[:, :, 0:cw], p['or_d'][:])
            nc.vector.tensor_add(s_nxt[:, :, 1 + c0:1 + c0 + cw],
                                 red[:, :, 0:cw],
                                 s_cur[:, :, 1 + c0:1 + c0 + cw])
            if li == depth - 1:
                fp2 = ck1.tile([DQ, 2, TC], F16, tag="p2", name="fp2")
                nc.scalar.activation(fp2[:, :, 0:cw],
                                     s_nxt[:, :, 1 + c0:1 + c0 + cw],
                                     AF.Square)
                fsq = pa.tile([1, TC], F32, tag="mm", name="fsq")
                for m in range(2):
                    nc.tensor.matmul(fsq[:, 0:cw], ones_ch[:],
                                     fp2[:, m, 0:cw],
                                     start=(m == 0), stop=(m == 1))
                frs = ck1.tile([1, TC], F16, tag="rstd", name="frs", bufs=1)
                nc.scalar.activation(frs[:, 0:cw], fsq[:, 0:cw], AF.Ln,
                                     bias=epsc[:], scale=1.0 / D_MODEL)
                finv = ck1.tile([1, TC], F16, tag="inv", name="finv", bufs=1)
                nc.scalar.activation(finv[:, 0:cw], frs[:, 0:cw], AF.Exp,
                                     scale=-0.5)
                fib = pa.tile([DQ, TC], F32, tag="mm", name="fib")
                nc.tensor.matmul(fib[:, 0:cw], ones_r[:], finv[:, 0:cw],
                                 start=True, stop=True)
                for m in range(2):
                    fn = ck1.tile([DQ, TC], F16, tag="fn", name=f"fn{m}",
                                  bufs=2)
                    nc.vector.tensor_mul(fn[:, 0:cw],
                                         s_nxt[:, m, 1 + c0:1 + c0 + cw],
                                         fib[:, 0:cw])
                    fo = ck1.tile([DQ, TC], F16, tag="fo", name=f"fo{m}",
                                  bufs=2)
                    nc.vector.tensor_scalar_mul(fo[:, 0:cw], fn[:, 0:cw],
                                                nfw[:, m:m + 1])
                    nc.sync.dma_start(out_d.ap()[m, :, c0:c0 + cw],
                                      fo[:, 0:cw])

        # -------- flat software-pipelined emission (depth-3) --------
        items = [(li, ci) for li in range(depth) for ci in range(NCH)]
        emitted_w = {}

        def w_for(li):
            if li not in emitted_w:
                emitted_w[li] = load_weights(li)
            return emitted_w[li]

        P = {}
        pend = None            # awaiting POST1
        pend2 = None
        pend3 = None           # awaiting POST2 (lag 3)
        hprev = None
        P[items[0]] = pre_mm(*items[0], w_for(items[0][0]))
        P[items[1]] = pre_mm(*items[1], w_for(items[1][0]))
        P[items[2]] = pre_mm(*items[2], w_for(items[2][0]))
        pre_bc1(*items[0], P[items[0]])
        pre_bc1(*items[1], P[items[1]])
        for i, (li, ci) in enumerate(items):
            p = P.pop((li, ci))
            pre_bc2(li, ci, p)
            if pend is not None:
                post1(*pend)
            if pend3 is not None:
                post2(*pend3)
            if i + 3 < len(items):
                P[items[i + 3]] = pre_mm(*items[i + 3],
                                         w_for(items[i + 3][0]))
            pre_dve_b(li, ci, p)
            hprev = scans(li, ci, p, hprev)
            if i + 2 < len(items):
                pre_bc1(*items[i + 2], P[items[i + 2]])
            pend3 = pend2
            pend2 = pend
            pend = (li, ci, p)
        post1(*pend)
        if pend3 is not None:
            post2(*pend3)
        post2(*pend2)
        post2(*pend)

    nc.compile()
    return nc


def _prep_inputs(inputs, depth=DEPTH):
    f = lambda k: np.asarray(inputs[k], np.float32)
    x = f("x")
    B = x.shape[0]
    lp_w, lp_b = f("lp_w"), f("lp_b")
    norm_w = f("norm_w")
    ipw = f("in_proj_w")
    conv_w, conv_b = f("conv_w"), f("conv_b")
    xpw = f("xproj_w")
    dt_w, dt_b = f("dt_w"), f("dt_b")
    A_log, D_ssm = f("A_log"), f("D_ssm")
    out_w = f("out_w")
    nfw = f("normf_w")
    proj_w, proj_b = f("proj_w"), f("proj_b")

    A_vals = -np.exp(A_log[:, 0, :]).astype(np.float32)

    h = np.einsum("bchw,dc->bdhw", x, proj_w) + proj_b[None, :, None, None]
    n_tok = x.shape[2] * x.shape[3]
    s0 = h.reshape(B, D_MODEL, n_tok).astype(np.float32)

    Wip = ipw * norm_w[:, None, :]

    W1 = lp_w[:, :, :D_MODEL]
    W2 = lp_w[:, :, D_MODEL:]
    W1p = W1 + W2
    W2p = -W2
    lpT = np.zeros((depth, DQ, 2, 2, 2, DQ), np.float32)
    for kh in range(2):
        for m in range(2):
            blk1 = W1p[:, m * DQ:(m + 1) * DQ, kh * DQ:(kh + 1) * DQ]
            blk2 = W2p[:, m * DQ:(m + 1) * DQ, kh * DQ:(kh + 1) * DQ]
            lpT[:, :, kh, 0, m, :] = blk1.transpose(0, 2, 1)
            lpT[:, :, kh, 1, m, :] = blk2.transpose(0, 2, 1)
    lpb = np.ascontiguousarray(lp_b.reshape(depth, 2, DQ).transpose(0, 2, 1))
    nfw2 = np.ascontiguousarray(nfw.reshape(2, DQ).T)

    in_maps = []
    for core in range(NCORES):
        b, q = core // 4, core % 4
        qsl = slice(q * DQ, (q + 1) * DQ)

        # own-quarter fused in_proj+conv weights:
        # cvip[li, e(part), kh, k, dcol] = Wip[li, q*96+dcol, kh*96+e]
        #                                  * conv_w[li, q*96+dcol, k]
        cvip = np.zeros((depth, DQ, 2, D_CONV, DQ), np.float32)
        for kh in range(2):
            wb = Wip[:, qsl, kh * DQ:(kh + 1) * DQ]     # (depth, d, e)
            for k in range(D_CONV):
                cvip[:, :, kh, k, :] = (
                    wb * conv_w[:, qsl, k][:, :, None]
                ).transpose(0, 2, 1)
        ipz = np.zeros((depth, DQ, 2, DQ), np.float32)
        for kh in range(2):
            ipz[:, :, kh, :] = Wip[:, D_INNER + q * DQ:D_INNER + (q + 1) * DQ,
                                   kh * DQ:(kh + 1) * DQ].transpose(0, 2, 1)
        cvb_own = conv_b[:, qsl]
        cvb = np.stack([cvb_own, -cvb_own], 2).astype(np.float32)
        xpT = np.ascontiguousarray(
            xpw[:, :, qsl].transpose(0, 2, 1))           # (depth, DQ, 44)
        dtw_q = dt_w[:, qsl, :]                       # (depth, 96, 12)
        dtb_q = dt_b[:, qsl]                          # (depth, 96)
        p_arr = np.arange(128)
        si_p, dd_p = p_arr // 32, p_arr % 32
        # packed dt matmul weights: dtT[li, r, j, p] = dtw_q[li, 32j+dd(p), r]
        dtT = np.zeros((depth, DT_RANK, 3, 128), np.float32)
        dtb_pk = np.zeros((depth, 128, 3), np.float32)
        for j in range(3):
            dtT[:, :, j, :] = dtw_q[:, 32 * j + dd_p, :].transpose(0, 2, 1)
            dtb_pk[:, :, j] = dtb_q[:, 32 * j + dd_p]
        Apk = np.zeros((depth, 128, 4), np.float32)
        for sg in range(4):
            Apk[:, :, sg] = A_vals[:, 4 * sg + si_p]
        Epk = (A_vals[:, si_p + 4] - A_vals[:, si_p])[:, :, None]
        red32 = (dd_p[:, None] == np.arange(32)[None, :]).astype(np.float16)
        dtT9 = np.ascontiguousarray(dtw_q.transpose(0, 2, 1))
        # own-quarter out projection: owT[li, dq(contract), m, e]
        #   = out_w[li, m*96+e, q*96+dq]
        owT = np.ascontiguousarray(
            out_w[:, :, qsl].transpose(0, 2, 1).reshape(depth, DQ, 2, DQ))

        in_maps.append({
            "s0": np.ascontiguousarray(
                s0[b].reshape(2, DQ, n_tok).transpose(1, 0, 2)
            ).astype(np.float16),
            "lpT": lpT.astype(np.float16),
            "lpb": lpb,
            "cvip": np.ascontiguousarray(cvip).astype(np.float16),
            "ipz": np.ascontiguousarray(ipz).astype(np.float16),
            "cv_b": np.ascontiguousarray(cvb),
            "xpT": xpT.astype(np.float16),
            "dtT": np.ascontiguousarray(dtT).astype(np.float16),
            "dt_b": np.ascontiguousarray(dtb_pk),
            "Apk": np.ascontiguousarray(Apk),
            "Epk": np.ascontiguousarray(Epk.astype(np.float32)),
            "red32": np.ascontiguousarray(red32),
            "dtT9": dtT9.astype(np.float16),
            "dtb9": np.ascontiguousarray(dtb_q[:, :, None].astype(np.float32)),
            "Dssm": np.ascontiguousarray(D_ssm[:, qsl, None]),
            "owT": owT.astype(np.float16),
            "nfw": nfw2,
        })
    return in_maps, A_vals, x.shape


def kernel(**inputs):
    in_maps, A_vals, xshape = _prep_inputs(inputs)
    key = ("v5", A_vals.tobytes())
    if key not in _CACHE:
        _CACHE[key] = _build(A_vals)
    nc = _CACHE[key]
    try:
        res = run_bass_kernel_spmd(nc, in_maps, core_ids=list(range(NCORES)))
    except Exception:
        res = run_bass_kernel_spmd(nc, in_maps, core_ids=list(range(NCORES)))
    B, _, H, W = xshape
    out = np.zeros((B, D_MODEL, H * W), np.float32)
    for b in range(B):
        r = res.results[b * 4]["out_s"]
        out[b, :DQ] = np.float32(r[0])
        out[b, DQ:] = np.float32(r[1])
    return out.reshape(B, D_MODEL, H, W)
